# revision 29
# baseline (speedup 1.0000x reference)
"""Causal dot-product attention (s=2048, b=4, h=16, d=128) on 8 TRN2 NeuronCores.

Sharding: batch*heads (64 pairs) split across 8 cores -> 8 (b,h) pairs per core.
Core c handles b = c // 2, heads h in [(c%2)*8, (c%2)*8 + 8).

Per-core kernel (Bass/Tile), per head:
  S^T[sk, sq] = K^T_j(stationary) . Q^T(moving)   (fp16 in, fp32 PSUM out)
  E = exp(S^T * 1/sqrt(d))  (ACT, fp16 out, grouped over pairs of j-blocks to
                             amortize the ~352-cycle ACTIVATE overhead; no
                             max-subtraction: scores ~ N(0,1) so exp is safe)
  causal: skip sk>sq blocks/columns; triangular fp16 mask multiply on the
          diagonal 128-wide subtile only (DVE)
  ctx[sq, 0:128] + rowsum[sq] (col 128) = sum_j E_j^T(stationary) . [V_j | 1]
  out = ctx * (1/rowsum)     (DVE reciprocal + per-partition scalar multiply)

Host-side layout prep: Q and K are transposed to [head, d, s], concatenated,
and cast to fp16 (fp32 matmuls cost two PE passes; fp16 also enables fast
weight load).  V is cast to fp16 with the softmax-denominator ones-column
baked in.  One DMA each per head -> single-semaphore waits (walrus rejects
instructions carrying >1 sync wait; Bacc legalizes the rest via event sems).
"""

import sys

if "/opt/trn_rl_repo" not in sys.path:
    sys.path.insert(0, "/opt/trn_rl_repo")

import numpy as np

import concourse.bacc as bacc
import concourse.bass as bass
import concourse.mybir as mybir
import concourse.tile as tile
from concourse.bass_utils import run_bass_kernel_spmd

S, B, H, D = 2048, 4, 16, 128
N_CORES = 8
HPC = (B * H) // N_CORES  # heads per core = 8
SCALE = 1.0 / float(np.sqrt(128.0))

SQ_BLK = 512  # sq block width per j-tile matmul
N_I = S // SQ_BLK  # 4 sq blocks per head
N_SK = S // 128  # 16 sk tiles per head
VW = 129  # V tile width incl. ones column


def build_nc():
    nc = bacc.Bacc()
    qk = nc.dram_tensor("qk", [HPC, D, 2 * S], mybir.dt.float16, kind="ExternalInput")
    v = nc.dram_tensor("v", [HPC, N_SK, 128, VW], mybir.dt.float16, kind="ExternalInput")
    out = nc.dram_tensor("out", [S, HPC * D], mybir.dt.float32, kind="ExternalOutput")

    with tile.TileContext(nc) as tc:
        with (
            tc.tile_pool(name="const", bufs=1) as constp,
            tc.tile_pool(name="qkp", bufs=2) as qkp,
            tc.tile_pool(name="vp", bufs=3) as vpool,
            tc.tile_pool(name="e", bufs=8) as ep,
            tc.tile_pool(name="stage", bufs=3) as stagep,
            tc.tile_pool(name="rec", bufs=8) as recp,
            tc.tile_pool(name="em", bufs=8) as emp,
            tc.tile_pool(name="ps_s", bufs=2, space="PSUM") as ps_s,
            tc.tile_pool(name="ps_c", bufs=1, space="PSUM") as ps_c,
        ):
            # tri[r, c] = 1.0 if c >= r else 0.0 (fp16) - diagonal-subtile mask
            tri = constp.tile([128, 128], mybir.dt.float16)
            nc.gpsimd.memset(tri[:], 1.0)
            nc.gpsimd.affine_select(
                out=tri[:],
                in_=tri[:],
                compare_op=mybir.AluOpType.is_ge,
                fill=0.0,
                base=0,
                pattern=[[1, 128]],
                channel_multiplier=-1,
            )
            # tiny dummy exp: triggers the one-time ~2.7us ACT table load
            # during the DMA prologue instead of before the first real exp
            warm = constp.tile([1, 8], mybir.dt.float32, name="warm")
            nc.scalar.activation(
                warm[:],
                tri[0:1, 0:8],
                mybir.ActivationFunctionType.Exp,
                scale=SCALE,
            )

            started_heads = set()
            started_i5 = set()
            vdummy_done = set()
            pending_epi = []
            ctx_holder = {}
            staged_holder = {}
            qk_holder = {}
            v_holder = {}

            def start_head(hh):
                qk_sb = qkp.tile([128, 2 * S], mybir.dt.float16, tag="qk", name="qk_sb")
                qk_holder[hh] = qk_sb
                v_sb = vpool.tile([128, N_SK * VW], mybir.dt.float16, tag="v", name="v_sb")
                v_holder[hh] = v_sb
                qk4 = qk_sb.rearrange("p (b c) -> p b c", c=SQ_BLK)
                qk4s = qk[hh, :, :].rearrange("p (b c) -> p b c", c=SQ_BLK)
                v3 = v_sb.rearrange("p (j e) -> p j e", e=VW)
                v3s = v[hh, :, :, :].rearrange("j p e -> p j e")
                if hh == 0:
                    # head 0 has no prefetch window: stream the data in
                    # compute order (q_b + k_b for block b, then its v tiles)
                    for b in range(N_I):
                        nc.sync.dma_start(
                            out=qk4[:, b :: N_I, :], in_=qk4s[:, b :: N_I, :]
                        )
                        nc.sync.dma_start(
                            out=v3[:, 4 * b : 4 * b + 4, :],
                            in_=v3s[:, 4 * b : 4 * b + 4, :],
                        )
                else:
                    # later heads are fully prefetched during the previous head
                    nc.sync.dma_start(out=qk_sb[:], in_=qk[hh, :, :])
                    nc.sync.dma_start(out=v3, in_=v3s)
                staged_holder[hh] = stagep.tile(
                    [128, N_SK * D], mybir.dt.float32, tag="o", name="staged"
                )

            def start_i5(hh, i5):
                ctx_ab = [
                    ps_c.tile(
                        [128, 2 * VW], mybir.dt.float32, tag=f"ctx{t}", name=f"ctx{t}"
                    )
                    for t in range(2)
                ]
                ctx_holder[(hh, i5)] = [
                    ctx_ab[tt // 2][:, (tt % 2) * VW : (tt % 2 + 1) * VW]
                    for tt in range(4)
                ]

            def i5_groups(i5):
                # up-to-3-j exp groups sharing one [128, 1536] psum tile.
                # quarter order is descending-c0 (highest-c0 diagonal tile
                # first, zero-c0 members behind), so the contiguous exp range
                # [c0_q0 : end] covers almost no causally-dead columns while
                # amortizing the ~352-cycle ACTIVATE overhead over 3 j-tiles.
                if i5 == 0:
                    return [[3, 1, 0], [2]]
                d3, d2, d1, d0 = (4 * i5 + k for k in (3, 2, 1, 0))
                f = list(range(4 * i5))
                gs = [[d3, d0, f.pop(0)], [d2, d1, f.pop(0)]]
                while f:
                    gs.append(f[:3])
                    f = f[3:]
                return gs

            def group_js(i5, g):
                return i5_groups(i5)[g]

            def emit_qk(hh, i5, p):
                if hh not in started_heads:
                    start_head(hh)
                    started_heads.add(hh)
                if hh + 1 < HPC and hh + 1 not in started_heads:
                    # issue the next head's DMAs a full head ahead
                    start_head(hh + 1)
                    started_heads.add(hh + 1)
                if (hh, i5) not in started_i5:
                    start_i5(hh, i5)
                    started_i5.add((hh, i5))
                qk_sb = qk_holder[hh]
                s_ps = ps_s.tile(
                    [128, 3 * SQ_BLK], mybir.dt.float32, tag="s", name="s_ps"
                )
                for q, j in enumerate(group_js(i5, p)):
                    t = j - 4 * i5
                    c0 = 128 * t if t > 0 else 0
                    nc.tensor.matmul(
                        s_ps[:, q * SQ_BLK + c0 : (q + 1) * SQ_BLK],
                        qk_sb[:, S + j * 128 : S + (j + 1) * 128],
                        qk_sb[:, i5 * SQ_BLK + c0 : (i5 + 1) * SQ_BLK],
                        start=True,
                        stop=True,
                    )
                return s_ps

            def emit_exp_pv(hh, i5, p, s_ps):
                qk_sb = qk_holder[hh]
                v_sb = v_holder[hh]
                ctx_t = ctx_holder[(hh, i5)]
                js = group_js(i5, p)
                t_q0 = js[0] - 4 * i5
                lo = 128 * t_q0 if t_q0 > 0 else 0  # first valid col of group
                hi = len(js) * SQ_BLK
                e_sb = ep.tile(
                    [128, 3 * SQ_BLK], mybir.dt.float16, tag="e", name="e_sb"
                )
                nc.scalar.activation(
                    e_sb[:, lo:hi],
                    s_ps[:, lo:hi],
                    mybir.ActivationFunctionType.Exp,
                    scale=SCALE,
                )
                if hh not in vdummy_done:
                    # absorb the v-DMA wait on PE right before the head's
                    # first PV matmul (scribbles on ctx, which the j=0
                    # start=True matmul then resets)
                    vdummy_done.add(hh)
                    nc.tensor.matmul(
                        ctx_t[0][0:1, 0:8],
                        v_sb[:, 0:1],
                        v_sb[:, 0:8],
                        start=True,
                        stop=True,
                        skip_group_check=True,
                    )
                # ascending-j emission keeps j=0's bank-clearing start=True
                # matmuls ahead of every other writer of the same psum bank
                for q, j in sorted(enumerate(js), key=lambda qj: qj[1]):
                    t = j - 4 * i5
                    c0 = 128 * t if t > 0 else 0
                    off = q * SQ_BLK
                    em = None
                    if t >= 0:
                        # masked diagonal subtile goes to its own tile so
                        # e_sb's slot release never waits on DVE
                        em = emp.tile(
                            [128, 128], mybir.dt.float16, tag="em", name="em"
                        )
                        nc.vector.tensor_mul(
                            em[:],
                            e_sb[:, off + c0 : off + c0 + 128],
                            tri[:],
                        )
                    # start=True clears the WHOLE psum bank, so only the
                    # bank-first accumulator (tt 0 / 2) may carry it; its
                    # bank-mate's first matmul relies on has_written=0 ->
                    # plain write semantics.
                    # per-tt last-emitted contributor across the i5's
                    # whole (ascending-j within group) emission sequence
                    seq = [jj for gg in i5_groups(i5) for jj in sorted(gg)]
                    stop_j = tuple(
                        [jj for jj in seq if jj <= 4 * i5 + tt][-1] for tt in range(4)
                    )
                    for tt in range(max(t, 0), 4):
                        lhs = (
                            em[:]
                            if (t >= 0 and tt == t)
                            else e_sb[:, off + tt * 128 : off + (tt + 1) * 128]
                        )
                        nc.tensor.matmul(
                            ctx_t[tt][:],
                            lhs,
                            v_sb[:, j * VW : (j + 1) * VW],
                            start=(j == 0 and tt % 2 == 0),
                            stop=(j == stop_j[tt]),
                            skip_group_check=True,
                        )
                # defer this i5's epilogue by one group so the NEXT group's
                # diagonal masks enter the DVE FIFO first (the 8-op epilogue
                # otherwise delays them at every i5/head seam)
                while pending_epi:
                    pending_epi.pop(0)()
                if p == len(i5_groups(i5)) - 1:

                    def epi(hh=hh, i5=i5, ctx_t=ctx_t):
                        staged = staged_holder[hh]
                        for tt in range(4):
                            rec = recp.tile(
                                [128, 1], mybir.dt.float32, tag="rec", name="rec"
                            )
                            nc.vector.reciprocal(rec[:], ctx_t[tt][:, 128:129])
                            nc.vector.tensor_scalar_mul(
                                staged[:, (i5 * 4 + tt) * D : (i5 * 4 + tt + 1) * D],
                                ctx_t[tt][:, 0:128],
                                rec[:],
                            )
                        nc.sync.dma_start(
                            out=out[
                                i5 * SQ_BLK : (i5 + 1) * SQ_BLK, hh * D : (hh + 1) * D
                            ].rearrange("(i p) d -> p i d", p=128),
                            in_=staged.rearrange("p (i d) -> p i d", d=D)[
                                :, i5 * 4 : (i5 + 1) * 4, :
                            ],
                        )

                    pending_epi.append(epi)

            groups = [
                (hh, i5, p)
                for hh in range(HPC)
                for i5 in range(N_I)
                for p in range(len(i5_groups(i5)))
            ]
            # one-group software-pipelined emission across ALL head/i5
            # boundaries: QK of group g+1 precedes exp+PV of group g in PE
            # program order, so PE never stalls behind the exp it feeds.
            prev = None
            for g in groups:
                s_ps = emit_qk(*g)
                if prev is not None:
                    emit_exp_pv(*prev[0], prev[1])
                prev = (g, s_ps)
            emit_exp_pv(*prev[0], prev[1])
            while pending_epi:
                pending_epi.pop(0)()
    nc.compile()
    return nc


_NC_CACHE = None


def _get_nc():
    global _NC_CACHE
    if _NC_CACHE is None:
        _NC_CACHE = build_nc()
    return _NC_CACHE


def _make_in_maps(query_layer, key_layer, value_layer):
    q = np.asarray(query_layer)
    k = np.asarray(key_layer)
    v = np.asarray(value_layer)
    in_maps = []
    for c in range(N_CORES):
        b = c // 2
        h0 = (c % 2) * HPC
        qkc = np.empty((HPC, D, 2 * S), dtype=np.float16)
        # [s, h, d] -> [h, d, s]
        qkc[:, :, :S] = q[:, b, h0 : h0 + HPC, :].transpose(1, 2, 0)
        qkc[:, :, S:] = k[:, b, h0 : h0 + HPC, :].transpose(1, 2, 0)
        # [s, h, d] -> [h, j, p, d] + ones column -> fp16
        vc = np.ones((HPC, N_SK, 128, VW), dtype=np.float16)
        vc[:, :, :, :D] = (
            v[:, b, h0 : h0 + HPC, :]
            .transpose(1, 0, 2)
            .reshape(HPC, N_SK, 128, D)
            .astype(np.float16)
        )
        in_maps.append({"qk": qkc, "v": vc})
    return in_maps


def run_spmd(in_maps, **kwargs):
    nc = _get_nc()
    return run_bass_kernel_spmd(nc, in_maps, core_ids=list(range(N_CORES)), **kwargs)


def kernel(query_layer, key_layer, value_layer):
    in_maps = _make_in_maps(query_layer, key_layer, value_layer)
    res = run_spmd(in_maps)
    full = np.empty((S, B, H * D), dtype=np.float32)
    for c in range(N_CORES):
        b = c // 2
        h0 = (c % 2) * HPC
        full[:, b, h0 * D : (h0 + HPC) * D] = res.results[c]["out"]
    return full


# revision 30
# speedup vs baseline: 1.0134x; 1.0134x over previous
"""Causal dot-product attention (s=2048, b=4, h=16, d=128) on 8 TRN2 NeuronCores.

Sharding: batch*heads (64 pairs) split across 8 cores -> 8 (b,h) pairs per core.
Core c handles b = c // 2, heads h in [(c%2)*8, (c%2)*8 + 8).

Per-core kernel (Bass/Tile), per head:
  S^T[sk, sq] = K^T_j(stationary) . Q^T(moving)   (fp16 in, fp32 PSUM out)
  E = exp(S^T * 1/sqrt(d))  (ACT, fp16 out, grouped over pairs of j-blocks to
                             amortize the ~352-cycle ACTIVATE overhead; no
                             max-subtraction: scores ~ N(0,1) so exp is safe)
  causal: skip sk>sq blocks/columns; triangular fp16 mask multiply on the
          diagonal 128-wide subtile only (DVE)
  ctx[sq, 0:128] + rowsum[sq] (col 128) = sum_j E_j^T(stationary) . [V_j | 1]
  out = ctx * (1/rowsum)     (DVE reciprocal + per-partition scalar multiply)

Host-side layout prep: Q and K are transposed to [head, d, s], concatenated,
and cast to fp16 (fp32 matmuls cost two PE passes; fp16 also enables fast
weight load).  V is cast to fp16 with the softmax-denominator ones-column
baked in.  One DMA each per head -> single-semaphore waits (walrus rejects
instructions carrying >1 sync wait; Bacc legalizes the rest via event sems).
"""

import sys

if "/opt/trn_rl_repo" not in sys.path:
    sys.path.insert(0, "/opt/trn_rl_repo")

import numpy as np

import concourse.bacc as bacc
import concourse.bass as bass
import concourse.mybir as mybir
import concourse.tile as tile
from concourse.bass_utils import run_bass_kernel_spmd

S, B, H, D = 2048, 4, 16, 128
N_CORES = 8
HPC = (B * H) // N_CORES  # heads per core = 8
SCALE = 1.0 / float(np.sqrt(128.0))

SQ_BLK = 512  # sq block width per j-tile matmul
N_I = S // SQ_BLK  # 4 sq blocks per head
N_SK = S // 128  # 16 sk tiles per head
VW = 129  # V tile width incl. ones column


def build_nc():
    nc = bacc.Bacc()
    qk = nc.dram_tensor("qk", [HPC, D, 2 * S], mybir.dt.float16, kind="ExternalInput")
    v = nc.dram_tensor("v", [HPC, N_SK, 128, VW], mybir.dt.float16, kind="ExternalInput")
    out = nc.dram_tensor("out", [S, HPC * D], mybir.dt.float32, kind="ExternalOutput")

    with tile.TileContext(nc) as tc:
        with (
            tc.tile_pool(name="const", bufs=1) as constp,
            tc.tile_pool(name="qkp", bufs=2) as qkp,
            tc.tile_pool(name="vp", bufs=3) as vpool,
            tc.tile_pool(name="e", bufs=8) as ep,
            tc.tile_pool(name="stage", bufs=3) as stagep,
            tc.tile_pool(name="rec", bufs=8) as recp,
            tc.tile_pool(name="em", bufs=8) as emp,
            tc.tile_pool(name="ps_s", bufs=2, space="PSUM") as ps_s,
            tc.tile_pool(name="ps_c", bufs=2, space="PSUM") as ps_c,
        ):
            # tri[r, c] = 1.0 if c >= r else 0.0 (fp16) - diagonal-subtile mask
            tri = constp.tile([128, 128], mybir.dt.float16)
            nc.gpsimd.memset(tri[:], 1.0)
            nc.gpsimd.affine_select(
                out=tri[:],
                in_=tri[:],
                compare_op=mybir.AluOpType.is_ge,
                fill=0.0,
                base=0,
                pattern=[[1, 128]],
                channel_multiplier=-1,
            )
            # tiny dummy exp: triggers the one-time ~2.7us ACT table load
            # during the DMA prologue instead of before the first real exp
            warm = constp.tile([1, 8], mybir.dt.float32, name="warm")
            nc.scalar.activation(
                warm[:],
                tri[0:1, 0:8],
                mybir.ActivationFunctionType.Exp,
                scale=SCALE,
            )

            started_heads = set()
            started_i5 = set()
            vdummy_done = set()
            pending_epi = []
            ctx_holder = {}
            staged_holder = {}
            qk_holder = {}
            v_holder = {}

            def start_head(hh):
                qk_sb = qkp.tile([128, 2 * S], mybir.dt.float16, tag="qk", name="qk_sb")
                qk_holder[hh] = qk_sb
                v_sb = vpool.tile([128, N_SK * VW], mybir.dt.float16, tag="v", name="v_sb")
                v_holder[hh] = v_sb
                qk4 = qk_sb.rearrange("p (b c) -> p b c", c=SQ_BLK)
                qk4s = qk[hh, :, :].rearrange("p (b c) -> p b c", c=SQ_BLK)
                v3 = v_sb.rearrange("p (j e) -> p j e", e=VW)
                v3s = v[hh, :, :, :].rearrange("j p e -> p j e")
                if hh == 0:
                    # head 0 has no prefetch window: stream the data in
                    # compute order (q_b + k_b for block b, then its v tiles)
                    for b in range(N_I):
                        nc.sync.dma_start(
                            out=qk4[:, b :: N_I, :], in_=qk4s[:, b :: N_I, :]
                        )
                        nc.sync.dma_start(
                            out=v3[:, 4 * b : 4 * b + 4, :],
                            in_=v3s[:, 4 * b : 4 * b + 4, :],
                        )
                else:
                    # later heads are fully prefetched during the previous head
                    nc.sync.dma_start(out=qk_sb[:], in_=qk[hh, :, :])
                    nc.sync.dma_start(out=v3, in_=v3s)
                staged_holder[hh] = stagep.tile(
                    [128, N_SK * D], mybir.dt.float32, tag="o", name="staged"
                )

            def start_i5(hh, i5):
                ctx_ab = [
                    ps_c.tile(
                        [128, 2 * VW], mybir.dt.float32, tag=f"ctx{t}", name=f"ctx{t}"
                    )
                    for t in range(2)
                ]
                ctx_holder[(hh, i5)] = [
                    ctx_ab[tt // 2][:, (tt % 2) * VW : (tt % 2 + 1) * VW]
                    for tt in range(4)
                ]

            def group_js(i5, g):
                # full groups pair adjacent j; the four diagonal j-tiles are
                # cross-paired (highest-c0 tile in quarter 0, lowest in
                # quarter 1) so the exp range [c0_q0 : 1024] skips almost all
                # causally-dead columns
                if g < 2 * i5:
                    return (2 * g, 2 * g + 1)
                if g == 2 * i5:
                    return (4 * i5 + 3, 4 * i5)
                return (4 * i5 + 2, 4 * i5 + 1)

            def emit_qk(hh, i5, p):
                if hh not in started_heads:
                    start_head(hh)
                    started_heads.add(hh)
                if hh + 1 < HPC and hh + 1 not in started_heads:
                    # issue the next head's DMAs a full head ahead
                    start_head(hh + 1)
                    started_heads.add(hh + 1)
                if (hh, i5) not in started_i5:
                    start_i5(hh, i5)
                    started_i5.add((hh, i5))
                qk_sb = qk_holder[hh]
                s_ps = ps_s.tile(
                    [128, 2 * SQ_BLK], mybir.dt.float32, tag="s", name="s_ps"
                )
                for q, j in enumerate(group_js(i5, p)):
                    t = j - 4 * i5
                    c0 = 128 * t if t > 0 else 0
                    nc.tensor.matmul(
                        s_ps[:, q * SQ_BLK + c0 : (q + 1) * SQ_BLK],
                        qk_sb[:, S + j * 128 : S + (j + 1) * 128],
                        qk_sb[:, i5 * SQ_BLK + c0 : (i5 + 1) * SQ_BLK],
                        start=True,
                        stop=True,
                    )
                return s_ps

            def emit_exp_pv(hh, i5, p, s_ps):
                qk_sb = qk_holder[hh]
                v_sb = v_holder[hh]
                ctx_t = ctx_holder[(hh, i5)]
                js = group_js(i5, p)
                t_q0 = js[0] - 4 * i5
                lo = 128 * t_q0 if t_q0 > 0 else 0  # first valid col of group
                e_sb = ep.tile(
                    [128, 2 * SQ_BLK], mybir.dt.float16, tag="e", name="e_sb"
                )
                nc.scalar.activation(
                    e_sb[:, lo : 2 * SQ_BLK],
                    s_ps[:, lo : 2 * SQ_BLK],
                    mybir.ActivationFunctionType.Exp,
                    scale=SCALE,
                )
                if hh not in vdummy_done:
                    # absorb the v-DMA wait on PE right before the head's
                    # first PV matmul (scribbles on ctx, which the j=0
                    # start=True matmul then resets)
                    vdummy_done.add(hh)
                    nc.tensor.matmul(
                        ctx_t[0][0:1, 0:8],
                        v_sb[:, 0:1],
                        v_sb[:, 0:8],
                        start=True,
                        stop=True,
                        skip_group_check=True,
                    )
                # ascending-j emission keeps j=0's bank-clearing start=True
                # matmuls ahead of every other writer of the same psum bank
                for q, j in sorted(enumerate(js), key=lambda qj: qj[1]):
                    t = j - 4 * i5
                    c0 = 128 * t if t > 0 else 0
                    off = q * SQ_BLK
                    em = None
                    if t >= 0:
                        # masked diagonal subtile goes to its own tile so
                        # e_sb's slot release never waits on DVE
                        em = emp.tile(
                            [128, 128], mybir.dt.float16, tag="em", name="em"
                        )
                        nc.vector.tensor_mul(
                            em[:],
                            e_sb[:, off + c0 : off + c0 + 128],
                            tri[:],
                        )
                    # start=True clears the WHOLE psum bank, so only the
                    # bank-first accumulator (tt 0 / 2) may carry it; its
                    # bank-mate's first matmul relies on has_written=0 ->
                    # plain write semantics.
                    # per-tt last-emitted contributor in ascending-j,
                    # cross-paired order: tt0 -> j=4*i5, tt1 -> +1, tt2/tt3 -> +2
                    stop_j = (4 * i5, 4 * i5 + 1, 4 * i5 + 2, 4 * i5 + 2)
                    for tt in range(max(t, 0), 4):
                        lhs = (
                            em[:]
                            if (t >= 0 and tt == t)
                            else e_sb[:, off + tt * 128 : off + (tt + 1) * 128]
                        )
                        nc.tensor.matmul(
                            ctx_t[tt][:],
                            lhs,
                            v_sb[:, j * VW : (j + 1) * VW],
                            start=(j == 0 and tt % 2 == 0),
                            stop=(j == stop_j[tt]),
                            skip_group_check=True,
                        )
                # defer this i5's epilogue by one group so the NEXT group's
                # diagonal masks enter the DVE FIFO first (the 8-op epilogue
                # otherwise delays them at every i5/head seam)
                while pending_epi:
                    pending_epi.pop(0)()
                if p == 2 * (i5 + 1) - 1:

                    def epi(hh=hh, i5=i5, ctx_t=ctx_t):
                        staged = staged_holder[hh]
                        for tt in range(4):
                            rec = recp.tile(
                                [128, 1], mybir.dt.float32, tag="rec", name="rec"
                            )
                            nc.vector.reciprocal(rec[:], ctx_t[tt][:, 128:129])
                            nc.vector.tensor_scalar_mul(
                                staged[:, (i5 * 4 + tt) * D : (i5 * 4 + tt + 1) * D],
                                ctx_t[tt][:, 0:128],
                                rec[:],
                            )
                        nc.sync.dma_start(
                            out=out[
                                i5 * SQ_BLK : (i5 + 1) * SQ_BLK, hh * D : (hh + 1) * D
                            ].rearrange("(i p) d -> p i d", p=128),
                            in_=staged.rearrange("p (i d) -> p i d", d=D)[
                                :, i5 * 4 : (i5 + 1) * 4, :
                            ],
                        )

                    pending_epi.append(epi)

            groups = [
                (hh, i5, p)
                for hh in range(HPC)
                for i5 in range(N_I)
                for p in range(2 * (i5 + 1))
            ]
            # one-group software-pipelined emission across ALL head/i5
            # boundaries: QK of group g+1 precedes exp+PV of group g in PE
            # program order, so PE never stalls behind the exp it feeds.
            prev = None
            for g in groups:
                s_ps = emit_qk(*g)
                if prev is not None:
                    emit_exp_pv(*prev[0], prev[1])
                prev = (g, s_ps)
            emit_exp_pv(*prev[0], prev[1])
            while pending_epi:
                pending_epi.pop(0)()
    nc.compile()
    return nc


_NC_CACHE = None


def _get_nc():
    global _NC_CACHE
    if _NC_CACHE is None:
        _NC_CACHE = build_nc()
    return _NC_CACHE


def _make_in_maps(query_layer, key_layer, value_layer):
    q = np.asarray(query_layer)
    k = np.asarray(key_layer)
    v = np.asarray(value_layer)
    in_maps = []
    for c in range(N_CORES):
        b = c // 2
        h0 = (c % 2) * HPC
        qkc = np.empty((HPC, D, 2 * S), dtype=np.float16)
        # [s, h, d] -> [h, d, s]
        qkc[:, :, :S] = q[:, b, h0 : h0 + HPC, :].transpose(1, 2, 0)
        qkc[:, :, S:] = k[:, b, h0 : h0 + HPC, :].transpose(1, 2, 0)
        # [s, h, d] -> [h, j, p, d] + ones column -> fp16
        vc = np.ones((HPC, N_SK, 128, VW), dtype=np.float16)
        vc[:, :, :, :D] = (
            v[:, b, h0 : h0 + HPC, :]
            .transpose(1, 0, 2)
            .reshape(HPC, N_SK, 128, D)
            .astype(np.float16)
        )
        in_maps.append({"qk": qkc, "v": vc})
    return in_maps


def run_spmd(in_maps, **kwargs):
    nc = _get_nc()
    return run_bass_kernel_spmd(nc, in_maps, core_ids=list(range(N_CORES)), **kwargs)


def kernel(query_layer, key_layer, value_layer):
    in_maps = _make_in_maps(query_layer, key_layer, value_layer)
    res = run_spmd(in_maps)
    full = np.empty((S, B, H * D), dtype=np.float32)
    for c in range(N_CORES):
        b = c // 2
        h0 = (c % 2) * HPC
        full[:, b, h0 * D : (h0 + HPC) * D] = res.results[c]["out"]
    return full
